# revision 11
# baseline (speedup 1.0000x reference)
"""MLA-style attention (nn_CausalSelfAttention_34626026341018) for 8 trn2 cores.

Shapes (hardcoded): B=4, T=2048, C=1024, H=16, HD=64, KV=64, QL=512.
Sharding: core c handles batch b=c//2, query half = c%2 (1024 queries), full
key range. Inputs are pre-transposed/rotated on host; every core runs the
identical program.

v1 design (vs v0 baseline):
- MLA absorb: y_h = (P_h @ c_kv) @ (w_proj_h @ wv_u_h).T so V is never
  materialized; wv_u/w_proj merge into Wc on host, bv_u folds into the
  output bias. The attention "PV" contraction outputs latent Z instead of v.
- Stage A in bf16 (halves x DMA), biases folded: b_qk' = w_qk@bq_d + b_qk.
- exp split across two engines: ACT does spline Exp for even pairs; DVE does
  a Schraudolph fast-exp for odd pairs via one tensor_scalar op
  (u16 = round(S*A + B), bitcast as fp16). The constant-factor part of the
  rounding bias cancels exactly in softmax.
- Rowsums of P via ones-matmuls 4-col-packed (tile_position (0,c)).
- Softmax normalization applied to Z (per head), broadcast of 1/D via
  K=1 fp32r matmuls; bv_u/b_proj applied once at the end (bb).
"""

import numpy as np
import ml_dtypes

import concourse.bass as bass
import concourse.mybir as mybir
import concourse.tile as tile
from concourse import bacc
from concourse.bass_utils import run_bass_kernel_spmd

F32 = mybir.dt.float32
F32R = mybir.dt.float32r
F16 = mybir.dt.float16
BF16 = mybir.dt.bfloat16
U16 = mybir.dt.uint16
AF = mybir.ActivationFunctionType
ALU = mybir.AluOpType

B, T, C = 4, 2048, 1024
H, HD, KV, QL = 16, 64, 64, 512
P = 128
TQ = 1024              # queries per core
KC = C // P            # 8 k-chunks over C
MQL = QL // P          # 4 m-tiles of c_q^T
KQL = QL // P          # 4 k-chunks over QL
MH = (H * KV) // P     # 8 m-tiles of q^T (= head pairs)
PAIRS = H // 2         # 8
TKT = T // P           # 16 s-chunks
NQ = TQ // 512         # 2 Tq chunks of 512

# Schraudolph fast-exp constants (fp16 bitcast, 0.125 score scale folded in)
EXP_A = float(0.125 * np.log2(np.e) * 1024.0)
EXP_B = float(1024.0 * (15.0 - 0.01))

_CACHE = {}


class _nullctx:
    def __enter__(self):
        return None

    def __exit__(self, *a):
        return False


def build_nc(reps=1, parts=('A', 'B', 'P'), noexp=False):
    nc = bacc.Bacc(None, target_bir_lowering=False)

    xT = nc.dram_tensor("xT", [C, T], BF16, kind="ExternalInput")
    wq_dT = nc.dram_tensor("wq_dT", [C, QL], BF16, kind="ExternalInput")
    w_qkT = nc.dram_tensor("w_qkT", [QL, H * KV], BF16, kind="ExternalInput")
    wkv_dT = nc.dram_tensor("wkv_dT", [C, KV], BF16, kind="ExternalInput")
    wcT = nc.dram_tensor("wcT", [H * KV, C], BF16, kind="ExternalInput")
    bqk_p = nc.dram_tensor("bqk_p", [P, MH], F32, kind="ExternalInput")
    bkv2 = nc.dram_tensor("bkv2", [P, 1], F32, kind="ExternalInput")
    bb_bc = nc.dram_tensor("bb_bc", [P, C], F32, kind="ExternalInput")
    ones16_d = nc.dram_tensor("ones16_d", [P, 1], F16, kind="ExternalInput")
    onesw_d = nc.dram_tensor("onesw_d", [P, 64], F16, kind="ExternalInput")
    ident_d = nc.dram_tensor("ident_d", [64, 64], F16, kind="ExternalInput")
    out = nc.dram_tensor("out", [TQ, C], F32, kind="ExternalOutput")

    with tile.TileContext(nc) as tc:
      with (tc.For_i(0, reps, 1) if reps > 1 else _nullctx()):
        with tc.tile_pool(name="persist", bufs=1) as pp:
            ones16 = pp.tile([P, 1], F16, name="ones16")
            onesw = pp.tile([P, 64], F16, name="onesw")
            ident = pp.tile([64, 64], F16, name="ident")
            bqk_sb = pp.tile([P, MH], F32, name="bqk_sb")
            bkv_sb = pp.tile([P, 1], F32, name="bkv_sb")
            bb_sb = pp.tile([P, C], F32, name="bb_sb")
            nc.sync.dma_start(ones16[:], ones16_d[:])
            nc.sync.dma_start(onesw[:], onesw_d[:])
            nc.sync.dma_start(ident[:], ident_d[:])
            nc.sync.dma_start(bqk_sb[:], bqk_p[:])
            nc.sync.dma_start(bkv_sb[:], bkv2[:])
            nc.sync.dma_start(bb_sb[:], bb_bc[:])

            ckv2 = pp.tile([P, T], F16, name="ckv2")        # c_kv^T dup'd
            ckv_nat = pp.tile([P, TKT * 65], F16, name="ckv_nat")    # [lat0..63, 1]
            q_sb = [pp.tile([P, TQ], F16, name=f"q{m}") for m in range(MH)]
            y_sb = [pp.tile([P, TQ], BF16, name=f"y{j}") for j in range(PAIRS)]
            wc_sb = [pp.tile([P, C], BF16, name=f"wc{k}") for k in range(KC)]

            # ---------------- stage A ----------------
            with tc.tile_pool(name="stA", bufs=1) as sa:
                xT_sb = [sa.tile([P, T], BF16, name=f"x{k}") for k in range(KC)]
                wq_sb = [sa.tile([P, QL], BF16, name=f"wq{k}") for k in range(KC)]
                wkv_sb = [sa.tile([P, KV], BF16, name=f"wkv{k}") for k in range(KC)]
                wqk_sb = [sa.tile([P, H * KV], BF16, name=f"wqk{k}")
                          for k in range(KQL)]
                cq_sb = [sa.tile([P, TQ], BF16, name=f"cq{m}") for m in range(MQL)]
                for k in range(KC):
                    nc.sync.dma_start(xT_sb[k][:], xT[k * P:(k + 1) * P, :])
                    nc.sync.dma_start(wkv_sb[k][:], wkv_dT[k * P:(k + 1) * P, :])
                    nc.sync.dma_start(wq_sb[k][:], wq_dT[k * P:(k + 1) * P, :])
                for k in range(KQL):
                    nc.sync.dma_start(wqk_sb[k][:], w_qkT[k * P:(k + 1) * P, :])
                for k in range(KC):
                    nc.sync.dma_start(wc_sb[k][:], wcT[k * P:(k + 1) * P, :])

                with (
                    tc.tile_pool(name="psA", bufs=2, space="PSUM") as psA,
                    tc.tile_pool(name="psT", bufs=2, space="PSUM") as psT,
                ):
                  if 'A' in parts:
                    # c_kv^T [64, T] in two 1024-col halves
                    for nh in range(2):
                        sl = slice(nh * 1024, (nh + 1) * 1024)
                        ckv_ps = psA.tile([64, 1024], F32, tag="ps", name="ckv_ps")
                        for ns in range(2):
                            ssl = slice(nh * 1024 + ns * 512,
                                        nh * 1024 + (ns + 1) * 512)
                            psl = slice(ns * 512, (ns + 1) * 512)
                            for k in range(KC):
                                nc.tensor.matmul(
                                    ckv_ps[:, psl], wkv_sb[k][:],
                                    xT_sb[k][:, ssl],
                                    start=(k == 0), stop=(k == KC - 1))
                        nc.scalar.activation(ckv2[0:64, sl], ckv_ps[:],
                                             AF.Identity, bias=bkv_sb[0:64, 0:1])
                    nc.sync.dma_start(ckv2[64:128, :], ckv2[0:64, :])

                    # ckv natural layout [T-chunk 128, KV+1] via PE transpose.
                    # The 65th (ones) column folds the softmax rowsum into the
                    # even heads' Z matmul (D lands on psum row 64).
                    nc.vector.memset(ckv_nat[:, 64:TKT * 65:65], 1.0)
                    for i in range(TKT):
                        tr_ps = psT.tile([P, KV], F16, tag="tr", name="tr_ps")
                        nc.tensor.transpose(
                            tr_ps[:], ckv2[0:64, i * P:(i + 1) * P], ident[:])
                        nc.vector.tensor_copy(
                            ckv_nat[:, i * 65:i * 65 + 64], tr_ps[:])

                    # c_q^T [QL, Tq] (queries = first TQ cols of rotated xT)
                    for m in range(MQL):
                        cq_ps = psA.tile([P, TQ], F32, tag="ps", name="cq_ps")
                        msl = slice(m * P, (m + 1) * P)
                        for n in range(NQ):
                            nsl = slice(n * 512, (n + 1) * 512)
                            for k in range(KC):
                                nc.tensor.matmul(
                                    cq_ps[:, nsl], wq_sb[k][:, msl],
                                    xT_sb[k][:, nsl],
                                    start=(k == 0), stop=(k == KC - 1))
                        nc.vector.tensor_copy(cq_sb[m][:], cq_ps[:])

                    # q^T [H*KV, Tq] with folded bias b_qk'
                    for m in range(MH):
                        q_ps = psA.tile([P, TQ], F32, tag="ps", name="q_ps")
                        msl = slice(m * P, (m + 1) * P)
                        for n in range(NQ):
                            nsl = slice(n * 512, (n + 1) * 512)
                            for k in range(KQL):
                                nc.tensor.matmul(
                                    q_ps[:, nsl], wqk_sb[k][:, msl],
                                    cq_sb[k][:, nsl],
                                    start=(k == 0), stop=(k == KQL - 1))
                        nc.scalar.activation(q_sb[m][:], q_ps[:], AF.Identity,
                                             bias=bqk_sb[:, m:m + 1])

            # ---------------- stage B ----------------
            with (
                tc.tile_pool(name="pPt", bufs=3) as pPt,
                tc.tile_pool(name="pR", bufs=2) as pR,
                tc.tile_pool(name="psS1", bufs=1, space="PSUM") as psS1,
                tc.tile_pool(name="psS2", bufs=1, space="PSUM") as psS2,
                tc.tile_pool(name="psZ", bufs=1, space="PSUM") as psZ,
                tc.tile_pool(name="psR", bufs=1, space="PSUM") as psR,
            ):
              if 'B' in parts:
                for blk in range(PAIRS // 2):
                    p1, p2 = 2 * blk, 2 * blk + 1
                    for n in range(NQ):
                        nsl = slice(n * 512, (n + 1) * 512)
                        # Zbig: 4 head-slices of [65used, 512] (row 64 = D)
                        Z = psZ.tile([P, 2048], F32, tag="Z", name="Z")
                        for i in range(TKT):
                            ssl = slice(i * P, (i + 1) * P)
                            S1 = psS1.tile([P, 1024], F32, tag="s1", name="S1")
                            S2 = psS2.tile([P, 1024], F32, tag="s2", name="S2")
                            nc.tensor.matmul(
                                S1[:, 0:512], ckv2[0:64, ssl],
                                q_sb[p1][0:64, nsl],
                                start=True, stop=True, tile_position=(0, 0))
                            nc.tensor.matmul(
                                S1[:, 512:1024], ckv2[64:128, ssl],
                                q_sb[p1][64:128, nsl],
                                start=True, stop=True, tile_position=(64, 0))
                            nc.tensor.matmul(
                                S2[:, 0:512], ckv2[0:64, ssl],
                                q_sb[p2][0:64, nsl],
                                start=True, stop=True, tile_position=(0, 0))
                            nc.tensor.matmul(
                                S2[:, 512:1024], ckv2[64:128, ssl],
                                q_sb[p2][64:128, nsl],
                                start=True, stop=True, tile_position=(64, 0))
                            Pt = pPt.tile([P, 2048], F16, tag="pt", name="Pt")
                            if not noexp:
                                nc.scalar.activation(Pt[:, 0:1024], S1[:],
                                                     AF.Exp, scale=0.125)
                                nc.vector.tensor_scalar(
                                    out=Pt[:, 1024:2048].bitcast(U16),
                                    in0=S2[:],
                                    scalar1=EXP_A, scalar2=EXP_B,
                                    op0=ALU.mult, op1=ALU.add)
                            else:
                                nc.vector.tensor_copy(
                                    Pt[:, 0:16], S1[:, 0:16])
                                nc.vector.tensor_copy(
                                    Pt[:, 1024:1040], S2[:, 0:16])
                            # Z accumulation (latent PV). Even heads carry
                            # the rowsum via the ones column (D at row 64);
                            # odd heads write rows 64:128 and their rowsum
                            # goes to the free rows 0:1 of their col range.
                            cka = ckv_nat[:, i * 65:(i + 1) * 65]
                            ck64 = ckv_nat[:, i * 65:i * 65 + 64]
                            for h4, psl in enumerate(
                                    (slice(0, 512), slice(512, 1024),
                                     slice(1024, 1536), slice(1536, 2048))):
                                if h4 % 2 == 0:
                                    nc.tensor.matmul(
                                        Z[0:65, psl], cka, Pt[:, psl],
                                        start=(i == 0), stop=(i == TKT - 1))
                                else:
                                    nc.tensor.matmul(
                                        Z[64:128, psl], ck64, Pt[:, psl],
                                        start=(i == 0), stop=(i == TKT - 1))
                                    nc.tensor.matmul(
                                        Z[0:1, psl], ones16[:, 0:1],
                                        Pt[:, psl],
                                        start=(i == 0), stop=(i == TKT - 1))
                        # D rows (0=odd, 64=even) -> SBUF fp16
                        d_sb = pR.tile([65, 2048], F16, tag="d_sb", name="d_sb")
                        nc.scalar.copy(d_sb[0:65, :], Z[0:65, :])
                        # broadcast D then reciprocal: Rb_sb = 1 / bcast(D)
                        for pi, pj in ((0, p1), (1, p2)):
                            Rb = psS1.tile([P, 512], F32, tag="s1", name="Rb")
                            nc.tensor.matmul(
                                Rb[0:64, :], onesw[64:65, :],
                                d_sb[64:65, 1024 * pi:1024 * pi + 512],
                                start=True, stop=True)
                            nc.tensor.matmul(
                                Rb[64:128, :], onesw[0:1, :],
                                d_sb[0:1, 1024 * pi + 512:1024 * (pi + 1)],
                                start=True, stop=True)
                            Rb_sb = pR.tile([P, 512], F32, tag="Rb_sb",
                                            name="Rb_sb")
                            nc.vector.reciprocal_approx_fast(Rb_sb[:], Rb[:])
                            nc.vector.tensor_tensor(
                                y_sb[pj][0:64, nsl],
                                Z[0:64, 1024 * pi:1024 * pi + 512],
                                Rb_sb[0:64, :], ALU.mult)
                            nc.vector.tensor_tensor(
                                y_sb[pj][64:128, nsl],
                                Z[64:128, 1024 * pi + 512:1024 * (pi + 1)],
                                Rb_sb[64:128, :], ALU.mult)

            # ---------------- proj ----------------
            with (
                tc.tile_pool(name="pO", bufs=2) as pO,
                tc.tile_pool(name="psO", bufs=2, space="PSUM") as psO,
            ):
              if 'P' in parts:
                for tt in range(TQ // P):
                    o_ps = psO.tile([P, C], F32, tag="o", name="o_ps")
                    tsl = slice(tt * P, (tt + 1) * P)
                    for n in range(C // 512):
                        nsl = slice(n * 512, (n + 1) * 512)
                        for k in range(KC):
                            nc.tensor.matmul(
                                o_ps[:, nsl], y_sb[k][:, tsl],
                                wc_sb[k][:, nsl],
                                start=(k == 0), stop=(k == KC - 1))
                    o_sb = pO.tile([P, C], F32, tag="o_sb", name="o_sb")
                    nc.vector.tensor_tensor(o_sb[:], o_ps[:], bb_sb[:], ALU.add)
                    nc.sync.dma_start(out[tsl, :], o_sb[:])

    nc.compile()
    return nc


def _prep_maps(x, wq_d, bq_d, w_qk, b_qk, wkv_d, bkv_d, wv_u, bv_u, w_proj, b_proj):
    f = np.float32
    bf = ml_dtypes.bfloat16
    x = np.asarray(x, f)
    wq_d, bq_d = np.asarray(wq_d, f), np.asarray(bq_d, f)
    w_qk, b_qk = np.asarray(w_qk, f), np.asarray(b_qk, f)
    wkv_d, bkv_d = np.asarray(wkv_d, f), np.asarray(bkv_d, f)
    wv_u, bv_u = np.asarray(wv_u, f), np.asarray(bv_u, f)
    w_proj, b_proj = np.asarray(w_proj, f), np.asarray(b_proj, f)

    wc = np.concatenate(
        [w_proj[:, h * HD:(h + 1) * HD] @ wv_u[h * HD:(h + 1) * HD, :]
         for h in range(H)], axis=1)                    # [C, H*KV]
    bb = b_proj + w_proj @ bv_u                          # [C]
    bqk_f = w_qk @ bq_d + b_qk                           # [H*KV]

    shared = {
        "wq_dT": np.ascontiguousarray(wq_d.T).astype(bf),
        "w_qkT": np.ascontiguousarray(w_qk.T).astype(bf),
        "wkv_dT": np.ascontiguousarray(wkv_d.T).astype(bf),
        "wcT": np.ascontiguousarray(wc.T).astype(bf),
        "bqk_p": np.ascontiguousarray(bqk_f.reshape(MH, P).T),
        "bkv2": np.concatenate([bkv_d, bkv_d]).reshape(P, 1).astype(f),
        "bb_bc": np.broadcast_to(bb, (P, C)).copy(),
        "ones16_d": np.ones((P, 1), np.float16),
        "onesw_d": np.ones((P, 64), np.float16),
        "ident_d": np.eye(64, dtype=np.float16),
    }
    in_maps = []
    for c in range(8):
        b, half = divmod(c, 2)
        xTb = np.ascontiguousarray(x[b].T)               # [C, T]
        if half:
            xTb = np.ascontiguousarray(
                np.concatenate([xTb[:, TQ:], xTb[:, :TQ]], axis=1))
        m = dict(shared)
        m["xT"] = xTb.astype(bf)
        in_maps.append(m)
    return in_maps


def kernel(**inputs):
    if "nc" not in _CACHE:
        _CACHE["nc"] = build_nc()
    nc = _CACHE["nc"]
    in_maps = _prep_maps(**inputs)
    res = run_bass_kernel_spmd(nc, in_maps, core_ids=list(range(8)))
    _CACHE["last_result"] = res
    y = np.empty((B, T, C), dtype=np.float32)
    for c in range(8):
        b, half = divmod(c, 2)
        y[b, half * TQ:(half + 1) * TQ, :] = res.results[c]["out"]
    return y


# revision 12
# speedup vs baseline: 1.4171x; 1.4171x over previous
"""MLA-style attention (nn_CausalSelfAttention_34626026341018) for 8 trn2 cores.

Shapes (hardcoded): B=4, T=2048, C=1024, H=16, HD=64, KV=64, QL=512.
Sharding: core c handles batch b=c//2, query half = c%2 (1024 queries), full
key range. Inputs are pre-transposed/rotated on host; every core runs the
identical program.

v1 design (vs v0 baseline):
- MLA absorb: y_h = (P_h @ c_kv) @ (w_proj_h @ wv_u_h).T so V is never
  materialized; wv_u/w_proj merge into Wc on host, bv_u folds into the
  output bias. The attention "PV" contraction outputs latent Z instead of v.
- Stage A in bf16 (halves x DMA), biases folded: b_qk' = w_qk@bq_d + b_qk.
- exp split across two engines: ACT does spline Exp for even pairs; DVE does
  a Schraudolph fast-exp for odd pairs via one tensor_scalar op
  (u16 = round(S*A + B), bitcast as fp16). The constant-factor part of the
  rounding bias cancels exactly in softmax.
- Rowsums of P via ones-matmuls 4-col-packed (tile_position (0,c)).
- Softmax normalization applied to Z (per head), broadcast of 1/D via
  K=1 fp32r matmuls; bv_u/b_proj applied once at the end (bb).
"""

import numpy as np
import ml_dtypes

import concourse.bass as bass
import concourse.mybir as mybir
import concourse.tile as tile
from concourse import bacc
from concourse.bass_utils import run_bass_kernel_spmd

F32 = mybir.dt.float32
F32R = mybir.dt.float32r
F16 = mybir.dt.float16
BF16 = mybir.dt.bfloat16
U16 = mybir.dt.uint16
AF = mybir.ActivationFunctionType
ALU = mybir.AluOpType

B, T, C = 4, 2048, 1024
H, HD, KV, QL = 16, 64, 64, 512
P = 128
TQ = 1024              # queries per core
KC = C // P            # 8 k-chunks over C
MQL = QL // P          # 4 m-tiles of c_q^T
KQL = QL // P          # 4 k-chunks over QL
MH = (H * KV) // P     # 8 m-tiles of q^T (= head pairs)
PAIRS = H // 2         # 8
TKT = T // P           # 16 s-chunks
NQ = TQ // 512         # 2 Tq chunks of 512

# Schraudolph fast-exp constants (fp16 bitcast, 0.125 score scale folded in)
EXP_A = float(0.125 * np.log2(np.e) * 1024.0)
EXP_B = float(1024.0 * (15.0 - 0.01))

_CACHE = {}


class _nullctx:
    def __enter__(self):
        return None

    def __exit__(self, *a):
        return False


def build_nc(reps=1, parts=('A', 'B', 'P'), noexp=False):
    nc = bacc.Bacc(None, target_bir_lowering=False)

    xT = nc.dram_tensor("xT", [C, T], BF16, kind="ExternalInput")
    wq_dT = nc.dram_tensor("wq_dT", [C, QL], BF16, kind="ExternalInput")
    w_qkT = nc.dram_tensor("w_qkT", [QL, H * KV], BF16, kind="ExternalInput")
    wkv_dT = nc.dram_tensor("wkv_dT", [C, KV], BF16, kind="ExternalInput")
    wcT = nc.dram_tensor("wcT", [H * KV, C], BF16, kind="ExternalInput")
    bqk_p = nc.dram_tensor("bqk_p", [P, MH], F32, kind="ExternalInput")
    bkv2 = nc.dram_tensor("bkv2", [P, 1], F32, kind="ExternalInput")
    bb_bc = nc.dram_tensor("bb_bc", [P, C], F32, kind="ExternalInput")
    ones16_d = nc.dram_tensor("ones16_d", [P, 1], F16, kind="ExternalInput")
    onesw_d = nc.dram_tensor("onesw_d", [P, 64], F16, kind="ExternalInput")
    ident_d = nc.dram_tensor("ident_d", [64, 64], F16, kind="ExternalInput")
    out = nc.dram_tensor("out", [TQ, C], F32, kind="ExternalOutput")

    with tile.TileContext(nc) as tc:
      with (tc.For_i(0, reps, 1) if reps > 1 else _nullctx()):
        with tc.tile_pool(name="persist", bufs=1) as pp:
            ones16 = pp.tile([P, 1], F16, name="ones16")
            onesw = pp.tile([P, 64], F16, name="onesw")
            ident = pp.tile([64, 64], F16, name="ident")
            bqk_sb = pp.tile([P, MH], F32, name="bqk_sb")
            bkv_sb = pp.tile([P, 1], F32, name="bkv_sb")
            bb_sb = pp.tile([P, C], F32, name="bb_sb")
            nc.sync.dma_start(ones16[:], ones16_d[:])
            nc.sync.dma_start(onesw[:], onesw_d[:])
            nc.sync.dma_start(ident[:], ident_d[:])
            nc.sync.dma_start(bqk_sb[:], bqk_p[:])
            nc.sync.dma_start(bkv_sb[:], bkv2[:])
            nc.sync.dma_start(bb_sb[:], bb_bc[:])

            ckv2 = pp.tile([P, T], F16, name="ckv2")        # c_kv^T dup'd
            ckv_nat = pp.tile([P, TKT * 65], F16, name="ckv_nat")    # [lat0..63, 1]
            q_sb = [pp.tile([P, TQ], F16, name=f"q{m}") for m in range(MH)]
            y_sb = [pp.tile([P, TQ], BF16, name=f"y{j}") for j in range(PAIRS)]
            wc_sb = [pp.tile([P, C], BF16, name=f"wc{k}") for k in range(KC)]

            # ---------------- stage A ----------------
            with tc.tile_pool(name="stA", bufs=1) as sa:
                xT_sb = [sa.tile([P, T], BF16, name=f"x{k}") for k in range(KC)]
                wq_sb = [sa.tile([P, QL], BF16, name=f"wq{k}") for k in range(KC)]
                wkv_sb = [sa.tile([P, KV], BF16, name=f"wkv{k}") for k in range(KC)]
                wqk_sb = [sa.tile([P, H * KV], BF16, name=f"wqk{k}")
                          for k in range(KQL)]
                cq_sb = [sa.tile([P, TQ], BF16, name=f"cq{m}") for m in range(MQL)]
                for k in range(KC):
                    nc.sync.dma_start(xT_sb[k][:], xT[k * P:(k + 1) * P, :])
                    nc.sync.dma_start(wkv_sb[k][:], wkv_dT[k * P:(k + 1) * P, :])
                    nc.sync.dma_start(wq_sb[k][:], wq_dT[k * P:(k + 1) * P, :])
                for k in range(KQL):
                    nc.sync.dma_start(wqk_sb[k][:], w_qkT[k * P:(k + 1) * P, :])
                for k in range(KC):
                    nc.sync.dma_start(wc_sb[k][:], wcT[k * P:(k + 1) * P, :])

                with (
                    tc.tile_pool(name="psA", bufs=2, space="PSUM") as psA,
                    tc.tile_pool(name="psT", bufs=2, space="PSUM") as psT,
                ):
                  if 'A' in parts:
                    # c_kv^T [64, T] in two 1024-col halves
                    for nh in range(2):
                        sl = slice(nh * 1024, (nh + 1) * 1024)
                        ckv_ps = psA.tile([64, 1024], F32, tag="ps", name="ckv_ps")
                        for ns in range(2):
                            ssl = slice(nh * 1024 + ns * 512,
                                        nh * 1024 + (ns + 1) * 512)
                            psl = slice(ns * 512, (ns + 1) * 512)
                            for k in range(KC):
                                nc.tensor.matmul(
                                    ckv_ps[:, psl], wkv_sb[k][:],
                                    xT_sb[k][:, ssl],
                                    start=(k == 0), stop=(k == KC - 1))
                        nc.scalar.activation(ckv2[0:64, sl], ckv_ps[:],
                                             AF.Identity, bias=bkv_sb[0:64, 0:1])
                    nc.sync.dma_start(ckv2[64:128, :], ckv2[0:64, :])

                    # ckv natural layout [T-chunk 128, KV+1] via PE transpose.
                    # The 65th (ones) column folds the softmax rowsum into the
                    # even heads' Z matmul (D lands on psum row 64).
                    nc.vector.memset(ckv_nat[:, 64:TKT * 65:65], 1.0)
                    for i in range(TKT):
                        tr_ps = psT.tile([P, KV], F16, tag="tr", name="tr_ps")
                        nc.tensor.transpose(
                            tr_ps[:], ckv2[0:64, i * P:(i + 1) * P], ident[:])
                        nc.vector.tensor_copy(
                            ckv_nat[:, i * 65:i * 65 + 64], tr_ps[:])

                    # c_q^T [QL, Tq] (queries = first TQ cols of rotated xT)
                    for m in range(MQL):
                        cq_ps = psA.tile([P, TQ], F32, tag="ps", name="cq_ps")
                        msl = slice(m * P, (m + 1) * P)
                        for n in range(NQ):
                            nsl = slice(n * 512, (n + 1) * 512)
                            for k in range(KC):
                                nc.tensor.matmul(
                                    cq_ps[:, nsl], wq_sb[k][:, msl],
                                    xT_sb[k][:, nsl],
                                    start=(k == 0), stop=(k == KC - 1))
                        nc.vector.tensor_copy(cq_sb[m][:], cq_ps[:])

                    # q^T [H*KV, Tq] with folded bias b_qk'
                    for m in range(MH):
                        q_ps = psA.tile([P, TQ], F32, tag="ps", name="q_ps")
                        msl = slice(m * P, (m + 1) * P)
                        for n in range(NQ):
                            nsl = slice(n * 512, (n + 1) * 512)
                            for k in range(KQL):
                                nc.tensor.matmul(
                                    q_ps[:, nsl], wqk_sb[k][:, msl],
                                    cq_sb[k][:, nsl],
                                    start=(k == 0), stop=(k == KQL - 1))
                        nc.scalar.activation(q_sb[m][:], q_ps[:], AF.Identity,
                                             bias=bqk_sb[:, m:m + 1])

            # ---------------- stage B ----------------
            with (
                tc.tile_pool(name="pPt", bufs=3) as pPt,
                tc.tile_pool(name="pR", bufs=2) as pR,
                tc.tile_pool(name="psS1", bufs=1, space="PSUM") as psS1,
                tc.tile_pool(name="psS2", bufs=1, space="PSUM") as psS2,
                tc.tile_pool(name="psZ", bufs=1, space="PSUM") as psZ,
                tc.tile_pool(name="psR", bufs=1, space="PSUM") as psR,
            ):
              if 'B' in parts:
                for blk in range(PAIRS // 2):
                    p1, p2 = 2 * blk, 2 * blk + 1
                    for n in range(NQ):
                        nsl = slice(n * 512, (n + 1) * 512)
                        Z = psZ.tile([P, 1024], F32, tag="Z", name="Z")
                        rs = psR.tile([P, 512], F32, tag="rs", name="rs")
                        pts = {}
                        # software-pipelined: iteration k issues S/exp for
                        # chunk k, then the Z/rs consumers for chunk k-1, so
                        # the PE never waits on the exp of the current chunk.
                        for k in range(TKT + 1):
                            if k < TKT:
                                ssl = slice(k * P, (k + 1) * P)
                                S1 = psS1.tile([P, 1024], F32, tag="s1",
                                               name="S1")
                                S2 = psS2.tile([P, 1024], F32, tag="s2",
                                               name="S2")
                                nc.tensor.matmul(
                                    S1[:, 0:512], ckv2[0:64, ssl],
                                    q_sb[p1][0:64, nsl],
                                    start=True, stop=True,
                                    tile_position=(0, 0))
                                nc.tensor.matmul(
                                    S1[:, 512:1024], ckv2[64:128, ssl],
                                    q_sb[p1][64:128, nsl],
                                    start=True, stop=True,
                                    tile_position=(64, 0))
                                nc.tensor.matmul(
                                    S2[:, 0:512], ckv2[0:64, ssl],
                                    q_sb[p2][0:64, nsl],
                                    start=True, stop=True,
                                    tile_position=(0, 0))
                                nc.tensor.matmul(
                                    S2[:, 512:1024], ckv2[64:128, ssl],
                                    q_sb[p2][64:128, nsl],
                                    start=True, stop=True,
                                    tile_position=(64, 0))
                                Pt = pPt.tile([P, 2048], F16, tag="pt",
                                              name="Pt")
                                pts[k] = Pt
                                if not noexp:
                                    nc.scalar.activation(Pt[:, 0:1024], S1[:],
                                                         AF.Exp, scale=0.125)
                                    nc.vector.tensor_scalar(
                                        out=Pt[:, 1024:2048].bitcast(U16),
                                        in0=S2[:],
                                        scalar1=EXP_A, scalar2=EXP_B,
                                        op0=ALU.mult, op1=ALU.add)
                                else:
                                    nc.vector.tensor_copy(
                                        Pt[:, 0:16], S1[:, 0:16])
                                    nc.vector.tensor_copy(
                                        Pt[:, 1024:1040], S2[:, 0:16])
                            if k >= 1:
                                i = k - 1
                                Pt = pts.pop(i)
                                ck = ckv_nat[:, i * 65:i * 65 + 64]
                                nc.tensor.matmul(
                                    Z[0:64, 0:512], ck, Pt[:, 0:512],
                                    start=(i == 0), stop=(i == TKT - 1),
                                    tile_position=(0, 0))
                                nc.tensor.matmul(
                                    Z[64:128, 0:512], ck, Pt[:, 512:1024],
                                    start=(i == 0), stop=(i == TKT - 1),
                                    tile_position=(0, 64))
                                nc.tensor.matmul(
                                    Z[0:64, 512:1024], ck, Pt[:, 1024:1536],
                                    start=(i == 0), stop=(i == TKT - 1),
                                    tile_position=(0, 0))
                                nc.tensor.matmul(
                                    Z[64:128, 512:1024], ck, Pt[:, 1536:2048],
                                    start=(i == 0), stop=(i == TKT - 1),
                                    tile_position=(0, 64))
                                for h4, psl in enumerate(
                                        (slice(0, 512), slice(512, 1024),
                                         slice(1024, 1536),
                                         slice(1536, 2048))):
                                    rp = 32 * h4
                                    nc.tensor.matmul(
                                        rs[rp:rp + 1, :], ones16[:, 0:1],
                                        Pt[:, psl],
                                        start=(i == 0), stop=(i == TKT - 1),
                                        tile_position=(0, rp))
                        # normalize: r = 1/D broadcast, Zn = Z * r
                        r32 = pR.tile([P, 512], F32, tag="r32", name="r32")
                        nc.vector.reciprocal_approx_fast(r32[0:97, :],
                                                         rs[0:97, :])
                        r_sb = pR.tile([P, 512], F16, tag="r_sb", name="r_sb")
                        nc.scalar.copy(r_sb[0:97, :], r32[0:97, :])
                        for pi, pj in ((0, p1), (1, p2)):
                            Rb = psR.tile([P, 512], F32, tag="Rb", name="Rb")
                            r0, r1 = 64 * pi, 64 * pi + 32
                            nc.tensor.matmul(
                                Rb[0:64, :], onesw[r0:r0 + 1, :],
                                r_sb[r0:r0 + 1, :],
                                start=True, stop=True, tile_position=(r0, 0))
                            nc.tensor.matmul(
                                Rb[64:128, :], onesw[r1:r1 + 1, :],
                                r_sb[r1:r1 + 1, :],
                                start=True, stop=True, tile_position=(r1, 64))
                            Rb_sb = pR.tile([P, 512], F32, tag="Rb_sb",
                                            name="Rb_sb")
                            nc.scalar.copy(Rb_sb[:], Rb[:])
                            nc.vector.tensor_tensor(
                                y_sb[pj][:, nsl], Z[:, 512 * pi:512 * (pi + 1)],
                                Rb_sb[:], ALU.mult)

            # ---------------- proj ----------------
            with (
                tc.tile_pool(name="pO", bufs=2) as pO,
                tc.tile_pool(name="psO", bufs=2, space="PSUM") as psO,
            ):
              if 'P' in parts:
                for tt in range(TQ // P):
                    o_ps = psO.tile([P, C], F32, tag="o", name="o_ps")
                    tsl = slice(tt * P, (tt + 1) * P)
                    for n in range(C // 512):
                        nsl = slice(n * 512, (n + 1) * 512)
                        for k in range(KC):
                            nc.tensor.matmul(
                                o_ps[:, nsl], y_sb[k][:, tsl],
                                wc_sb[k][:, nsl],
                                start=(k == 0), stop=(k == KC - 1))
                    o_sb = pO.tile([P, C], F32, tag="o_sb", name="o_sb")
                    nc.vector.tensor_tensor(o_sb[:], o_ps[:], bb_sb[:], ALU.add)
                    nc.sync.dma_start(out[tsl, :], o_sb[:])

    nc.compile()
    return nc


def _prep_maps(x, wq_d, bq_d, w_qk, b_qk, wkv_d, bkv_d, wv_u, bv_u, w_proj, b_proj):
    f = np.float32
    bf = ml_dtypes.bfloat16
    x = np.asarray(x, f)
    wq_d, bq_d = np.asarray(wq_d, f), np.asarray(bq_d, f)
    w_qk, b_qk = np.asarray(w_qk, f), np.asarray(b_qk, f)
    wkv_d, bkv_d = np.asarray(wkv_d, f), np.asarray(bkv_d, f)
    wv_u, bv_u = np.asarray(wv_u, f), np.asarray(bv_u, f)
    w_proj, b_proj = np.asarray(w_proj, f), np.asarray(b_proj, f)

    wc = np.concatenate(
        [w_proj[:, h * HD:(h + 1) * HD] @ wv_u[h * HD:(h + 1) * HD, :]
         for h in range(H)], axis=1)                    # [C, H*KV]
    bb = b_proj + w_proj @ bv_u                          # [C]
    bqk_f = w_qk @ bq_d + b_qk                           # [H*KV]

    shared = {
        "wq_dT": np.ascontiguousarray(wq_d.T).astype(bf),
        "w_qkT": np.ascontiguousarray(w_qk.T).astype(bf),
        "wkv_dT": np.ascontiguousarray(wkv_d.T).astype(bf),
        "wcT": np.ascontiguousarray(wc.T).astype(bf),
        "bqk_p": np.ascontiguousarray(bqk_f.reshape(MH, P).T),
        "bkv2": np.concatenate([bkv_d, bkv_d]).reshape(P, 1).astype(f),
        "bb_bc": np.broadcast_to(bb, (P, C)).copy(),
        "ones16_d": np.ones((P, 1), np.float16),
        "onesw_d": np.ones((P, 64), np.float16),
        "ident_d": np.eye(64, dtype=np.float16),
    }
    in_maps = []
    for c in range(8):
        b, half = divmod(c, 2)
        xTb = np.ascontiguousarray(x[b].T)               # [C, T]
        if half:
            xTb = np.ascontiguousarray(
                np.concatenate([xTb[:, TQ:], xTb[:, :TQ]], axis=1))
        m = dict(shared)
        m["xT"] = xTb.astype(bf)
        in_maps.append(m)
    return in_maps


def kernel(**inputs):
    if "nc" not in _CACHE:
        _CACHE["nc"] = build_nc()
    nc = _CACHE["nc"]
    in_maps = _prep_maps(**inputs)
    res = run_bass_kernel_spmd(nc, in_maps, core_ids=list(range(8)))
    _CACHE["last_result"] = res
    y = np.empty((B, T, C), dtype=np.float32)
    for c in range(8):
        b, half = divmod(c, 2)
        y[b, half * TQ:(half + 1) * TQ, :] = res.results[c]["out"]
    return y


# revision 17
# speedup vs baseline: 1.5054x; 1.0624x over previous
"""MLA-style attention (nn_CausalSelfAttention_34626026341018) for 8 trn2 cores.

Shapes (hardcoded): B=4, T=2048, C=1024, H=16, HD=64, KV=64, QL=512.
Sharding: core c handles batch b=c//2, query half = c%2 (1024 queries), full
key range. Inputs are pre-transposed/rotated on host; every core runs the
identical program.

v1 design (vs v0 baseline):
- MLA absorb: y_h = (P_h @ c_kv) @ (w_proj_h @ wv_u_h).T so V is never
  materialized; wv_u/w_proj merge into Wc on host, bv_u folds into the
  output bias. The attention "PV" contraction outputs latent Z instead of v.
- Stage A in bf16 (halves x DMA), biases folded: b_qk' = w_qk@bq_d + b_qk.
- exp split across two engines: ACT does spline Exp for even pairs; DVE does
  a Schraudolph fast-exp for odd pairs via one tensor_scalar op
  (u16 = round(S*A + B), bitcast as fp16). The constant-factor part of the
  rounding bias cancels exactly in softmax.
- Rowsums of P via ones-matmuls 4-col-packed (tile_position (0,c)).
- Softmax normalization applied to Z (per head), broadcast of 1/D via
  K=1 fp32r matmuls; bv_u/b_proj applied once at the end (bb).
"""

import numpy as np
import ml_dtypes

import concourse.bass as bass
import concourse.mybir as mybir
import concourse.tile as tile
from concourse import bacc
from concourse.bass_utils import run_bass_kernel_spmd

F32 = mybir.dt.float32
F32R = mybir.dt.float32r
F16 = mybir.dt.float16
BF16 = mybir.dt.bfloat16
U16 = mybir.dt.uint16
AF = mybir.ActivationFunctionType
ALU = mybir.AluOpType

B, T, C = 4, 2048, 1024
H, HD, KV, QL = 16, 64, 64, 512
P = 128
TQ = 1024              # queries per core
KC = C // P            # 8 k-chunks over C
MQL = QL // P          # 4 m-tiles of c_q^T
KQL = QL // P          # 4 k-chunks over QL
MH = (H * KV) // P     # 8 m-tiles of q^T (= head pairs)
PAIRS = H // 2         # 8
TKT = T // P           # 16 s-chunks
NQ = TQ // 512         # 2 Tq chunks of 512

# Schraudolph fast-exp constants (fp16 bitcast, 0.125 score scale folded in)
EXP_A = float(0.125 * np.log2(np.e) * 1024.0)
EXP_B = float(1024.0 * (15.0 - 0.01))

_CACHE = {}


class _nullctx:
    def __enter__(self):
        return None

    def __exit__(self, *a):
        return False



def _dedup_ldweights(nc):
    """Shrink duplicate weight loads to one column.

    The toolchain splits every matmul into Ldweights+Matmult with no dedup,
    so reloading identical weights into the same PE tile position pays the
    full serial load each time. Re-loading only column 0 (same data) leaves
    the remaining array cells -- already holding that exact data -- intact,
    while keeping the instruction (and its semaphore waits) in place.
    Invalidate tracked positions on any tiling-mode change or non-matmul PE
    array activity.
    """
    import concourse.mybir as _mb
    n = 0
    for blk in nc.m.functions[0].blocks:
        last = {}
        last_mode = None
        for ins in blk.instructions:
            if getattr(ins, "engine", None) != _mb.EngineType.PE:
                continue
            tn = type(ins).__name__
            if tn == "InstLdweights":
                ap = ins.ins[0]
                mode = tuple(ins.tile_size) if ins.tile_size else None
                if mode != last_mode:
                    last.clear()
                    last_mode = mode
                pos = tuple(ins.tile_position) if ins.tile_position else (0, 0)
                sig = (ap.memref, ap.offset, str(ap.ap), str(ap.dtype))
                if last.get(pos) == sig:
                    pat = [list(d) for d in ap.ap]
                    if pat and pat[-1][1] > 1:
                        pat[-1] = [pat[-1][0], 1]
                        ap.ap = pat
                        n += 1
                else:
                    last[pos] = sig
            elif tn == "InstMatmult" and not ins.is_transpose:
                mode = tuple(ins.tile_size) if ins.tile_size else None
                if mode != last_mode:
                    last.clear()
                    last_mode = mode
            elif tn == "InstEventSemaphore":
                pass
            else:
                last.clear()
                last_mode = None
    return n

def build_nc(reps=1, parts=('A', 'B', 'P'), noexp=False):
    nc = bacc.Bacc(None, target_bir_lowering=False)

    xT = nc.dram_tensor("xT", [C, T], BF16, kind="ExternalInput")
    wq_dT = nc.dram_tensor("wq_dT", [C, QL], BF16, kind="ExternalInput")
    w_qkT = nc.dram_tensor("w_qkT", [QL, H * KV], BF16, kind="ExternalInput")
    wkv_dT = nc.dram_tensor("wkv_dT", [C, KV], BF16, kind="ExternalInput")
    wcT = nc.dram_tensor("wcT", [H * KV, C], BF16, kind="ExternalInput")
    bqk_p = nc.dram_tensor("bqk_p", [P, MH], F32, kind="ExternalInput")
    bkv2 = nc.dram_tensor("bkv2", [P, 1], F32, kind="ExternalInput")
    bb_bc = nc.dram_tensor("bb_bc", [P, C], F32, kind="ExternalInput")
    ones16_d = nc.dram_tensor("ones16_d", [P, 1], F16, kind="ExternalInput")
    onesw_d = nc.dram_tensor("onesw_d", [P, 64], F16, kind="ExternalInput")
    ident_d = nc.dram_tensor("ident_d", [64, 64], F16, kind="ExternalInput")
    out = nc.dram_tensor("out", [TQ, C], F32, kind="ExternalOutput")

    with tile.TileContext(nc) as tc:
      with (tc.For_i(0, reps, 1) if reps > 1 else _nullctx()):
        with tc.tile_pool(name="persist", bufs=1) as pp:
            ones16 = pp.tile([P, 1], F16, name="ones16")
            onesw = pp.tile([P, 64], F16, name="onesw")
            ident = pp.tile([64, 64], F16, name="ident")
            bqk_sb = pp.tile([P, MH], F32, name="bqk_sb")
            bkv_sb = pp.tile([P, 1], F32, name="bkv_sb")
            bb_sb = pp.tile([P, C], F32, name="bb_sb")
            nc.sync.dma_start(ones16[:], ones16_d[:])
            nc.sync.dma_start(onesw[:], onesw_d[:])
            nc.sync.dma_start(ident[:], ident_d[:])
            nc.sync.dma_start(bqk_sb[:], bqk_p[:])
            nc.sync.dma_start(bkv_sb[:], bkv2[:])
            nc.sync.dma_start(bb_sb[:], bb_bc[:])

            ckv2 = pp.tile([P, T], F16, name="ckv2")        # c_kv^T dup'd
            ckv_nat = pp.tile([P, TKT * 65], F16, name="ckv_nat")    # [lat0..63, 1]
            q_sb = [pp.tile([P, TQ], F16, name=f"q{m}") for m in range(MH)]
            y_sb = [pp.tile([P, TQ], BF16, name=f"y{j}") for j in range(PAIRS)]
            wc_sb = [pp.tile([P, C], BF16, name=f"wc{k}") for k in range(KC)]

            # ---------------- stage A ----------------
            with tc.tile_pool(name="stA", bufs=1) as sa:
                xT_sb = [sa.tile([P, T], BF16, name=f"x{k}") for k in range(KC)]
                wq_sb = [sa.tile([P, QL], BF16, name=f"wq{k}") for k in range(KC)]
                wkv_sb = [sa.tile([P, KV], BF16, name=f"wkv{k}") for k in range(KC)]
                wqk_sb = [sa.tile([P, H * KV], BF16, name=f"wqk{k}")
                          for k in range(KQL)]
                cq_sb = [sa.tile([P, TQ], BF16, name=f"cq{m}") for m in range(MQL)]
                for k in range(KC):
                    nc.sync.dma_start(xT_sb[k][:], xT[k * P:(k + 1) * P, :])
                    nc.sync.dma_start(wkv_sb[k][:], wkv_dT[k * P:(k + 1) * P, :])
                    nc.sync.dma_start(wq_sb[k][:], wq_dT[k * P:(k + 1) * P, :])
                for k in range(KQL):
                    nc.sync.dma_start(wqk_sb[k][:], w_qkT[k * P:(k + 1) * P, :])
                for k in range(KC):
                    nc.sync.dma_start(wc_sb[k][:], wcT[k * P:(k + 1) * P, :])

                with (
                    tc.tile_pool(name="psA", bufs=2, space="PSUM") as psA,
                    tc.tile_pool(name="psT", bufs=2, space="PSUM") as psT,
                ):
                  if 'A' in parts:
                    # c_kv^T [64, T] in two 1024-col halves
                    for nh in range(2):
                        sl = slice(nh * 1024, (nh + 1) * 1024)
                        ckv_ps = psA.tile([64, 1024], F32, tag="ps", name="ckv_ps")
                        for ns in range(2):
                            ssl = slice(nh * 1024 + ns * 512,
                                        nh * 1024 + (ns + 1) * 512)
                            psl = slice(ns * 512, (ns + 1) * 512)
                            for k in range(KC):
                                nc.tensor.matmul(
                                    ckv_ps[:, psl], wkv_sb[k][:],
                                    xT_sb[k][:, ssl],
                                    start=(k == 0), stop=(k == KC - 1))
                        nc.scalar.activation(ckv2[0:64, sl], ckv_ps[:],
                                             AF.Identity, bias=bkv_sb[0:64, 0:1])
                    nc.sync.dma_start(ckv2[64:128, :], ckv2[0:64, :])

                    # ckv natural layout [T-chunk 128, KV+1] via PE transpose.
                    # The 65th (ones) column folds the softmax rowsum into the
                    # even heads' Z matmul (D lands on psum row 64).
                    nc.vector.memset(ckv_nat[:, 64:TKT * 65:65], 1.0)
                    for i in range(TKT):
                        tr_ps = psT.tile([P, KV], F16, tag="tr", name="tr_ps")
                        nc.tensor.transpose(
                            tr_ps[:], ckv2[0:64, i * P:(i + 1) * P], ident[:])
                        nc.vector.tensor_copy(
                            ckv_nat[:, i * 65:i * 65 + 64], tr_ps[:])

                    # c_q^T [QL, Tq] (queries = first TQ cols of rotated xT)
                    for m in range(MQL):
                        cq_ps = psA.tile([P, TQ], F32, tag="ps", name="cq_ps")
                        msl = slice(m * P, (m + 1) * P)
                        for k in range(KC):
                            for n in range(NQ):
                                nsl = slice(n * 512, (n + 1) * 512)
                                nc.tensor.matmul(
                                    cq_ps[:, nsl], wq_sb[k][:, msl],
                                    xT_sb[k][:, nsl],
                                    start=(k == 0), stop=(k == KC - 1))
                        nc.vector.tensor_copy(cq_sb[m][:], cq_ps[:])

                    # q^T [H*KV, Tq] with folded bias b_qk'
                    for m in range(MH):
                        q_ps = psA.tile([P, TQ], F32, tag="ps", name="q_ps")
                        msl = slice(m * P, (m + 1) * P)
                        for k in range(KQL):
                            for n in range(NQ):
                                nsl = slice(n * 512, (n + 1) * 512)
                                nc.tensor.matmul(
                                    q_ps[:, nsl], wqk_sb[k][:, msl],
                                    cq_sb[k][:, nsl],
                                    start=(k == 0), stop=(k == KQL - 1))
                        nc.scalar.activation(q_sb[m][:], q_ps[:], AF.Identity,
                                             bias=bqk_sb[:, m:m + 1])

            # ---------------- stage B ----------------
            with (
                tc.tile_pool(name="pPt", bufs=3) as pPt,
                tc.tile_pool(name="pR", bufs=2) as pR,
                tc.tile_pool(name="psS1", bufs=1, space="PSUM") as psS1,
                tc.tile_pool(name="psS2", bufs=1, space="PSUM") as psS2,
                tc.tile_pool(name="psZ", bufs=1, space="PSUM") as psZ,
                tc.tile_pool(name="psR", bufs=1, space="PSUM") as psR,
            ):
              if 'B' in parts:
                for blk in range(PAIRS // 2):
                    p1, p2 = 2 * blk, 2 * blk + 1
                    for n in range(NQ):
                        nsl = slice(n * 512, (n + 1) * 512)
                        Z = psZ.tile([P, 1024], F32, tag="Z", name="Z")
                        rs = psR.tile([P, 512], F32, tag="rs", name="rs")
                        pts = {}
                        # software-pipelined: iteration k issues S/exp for
                        # chunk k, then the Z/rs consumers for chunk k-1, so
                        # the PE never waits on the exp of the current chunk.
                        for k in range(TKT + 1):
                            if k < TKT:
                                ssl = slice(k * P, (k + 1) * P)
                                S1 = psS1.tile([P, 1024], F32, tag="s1",
                                               name="S1")
                                S2 = psS2.tile([P, 1024], F32, tag="s2",
                                               name="S2")
                                nc.tensor.matmul(
                                    S1[:, 0:512], ckv2[0:64, ssl],
                                    q_sb[p1][0:64, nsl],
                                    start=True, stop=True,
                                    tile_position=(0, 0))
                                nc.tensor.matmul(
                                    S1[:, 512:1024], ckv2[64:128, ssl],
                                    q_sb[p1][64:128, nsl],
                                    start=True, stop=True,
                                    tile_position=(64, 0))
                                nc.tensor.matmul(
                                    S2[:, 0:512], ckv2[0:64, ssl],
                                    q_sb[p2][0:64, nsl],
                                    start=True, stop=True,
                                    tile_position=(0, 0))
                                nc.tensor.matmul(
                                    S2[:, 512:1024], ckv2[64:128, ssl],
                                    q_sb[p2][64:128, nsl],
                                    start=True, stop=True,
                                    tile_position=(64, 0))
                                Pt = pPt.tile([P, 2048], F16, tag="pt",
                                              name="Pt")
                                pts[k] = Pt
                                if not noexp:
                                    nc.scalar.activation(Pt[:, 0:1024], S1[:],
                                                         AF.Exp, scale=0.125)
                                    nc.vector.tensor_scalar(
                                        out=Pt[:, 1024:2048].bitcast(U16),
                                        in0=S2[:],
                                        scalar1=EXP_A, scalar2=EXP_B,
                                        op0=ALU.mult, op1=ALU.add)
                                else:
                                    nc.vector.tensor_copy(
                                        Pt[:, 0:16], S1[:, 0:16])
                                    nc.vector.tensor_copy(
                                        Pt[:, 1024:1040], S2[:, 0:16])
                            if k >= 1:
                                i = k - 1
                                Pt = pts.pop(i)
                                ck = ckv_nat[:, i * 65:i * 65 + 64]
                                nc.tensor.matmul(
                                    Z[0:64, 0:512], ck, Pt[:, 0:512],
                                    start=(i == 0), stop=(i == TKT - 1),
                                    tile_position=(0, 0))
                                nc.tensor.matmul(
                                    Z[64:128, 0:512], ck, Pt[:, 512:1024],
                                    start=(i == 0), stop=(i == TKT - 1),
                                    tile_position=(0, 64))
                                nc.tensor.matmul(
                                    Z[0:64, 512:1024], ck, Pt[:, 1024:1536],
                                    start=(i == 0), stop=(i == TKT - 1),
                                    tile_position=(0, 0))
                                nc.tensor.matmul(
                                    Z[64:128, 512:1024], ck, Pt[:, 1536:2048],
                                    start=(i == 0), stop=(i == TKT - 1),
                                    tile_position=(0, 64))
                                for h4, psl in enumerate(
                                        (slice(0, 512), slice(512, 1024),
                                         slice(1024, 1536),
                                         slice(1536, 2048))):
                                    rp = 32 * h4
                                    nc.tensor.matmul(
                                        rs[rp:rp + 1, :], ones16[:, 0:1],
                                        Pt[:, psl],
                                        start=(i == 0), stop=(i == TKT - 1),
                                        tile_position=(0, rp))
                        # normalize: r = 1/D broadcast, Zn = Z * r
                        r32 = pR.tile([P, 512], F32, tag="r32", name="r32")
                        nc.vector.reciprocal_approx_fast(r32[0:97, :],
                                                         rs[0:97, :])
                        r_sb = pR.tile([P, 512], F16, tag="r_sb", name="r_sb")
                        nc.scalar.copy(r_sb[0:97, :], r32[0:97, :])
                        for pi, pj in ((0, p1), (1, p2)):
                            Rb = psR.tile([P, 512], F32, tag="Rb", name="Rb")
                            r0, r1 = 64 * pi, 64 * pi + 32
                            nc.tensor.matmul(
                                Rb[0:64, :], onesw[r0:r0 + 1, :],
                                r_sb[r0:r0 + 1, :],
                                start=True, stop=True, tile_position=(r0, 0))
                            nc.tensor.matmul(
                                Rb[64:128, :], onesw[r1:r1 + 1, :],
                                r_sb[r1:r1 + 1, :],
                                start=True, stop=True, tile_position=(r1, 64))
                            Rb_sb = pR.tile([P, 512], F32, tag="Rb_sb",
                                            name="Rb_sb")
                            nc.scalar.copy(Rb_sb[:], Rb[:])
                            nc.vector.tensor_tensor(
                                y_sb[pj][:, nsl], Z[:, 512 * pi:512 * (pi + 1)],
                                Rb_sb[:], ALU.mult)

            # ---------------- proj ----------------
            with (
                tc.tile_pool(name="pO", bufs=2) as pO,
                tc.tile_pool(name="psO", bufs=2, space="PSUM") as psO,
            ):
              if 'P' in parts:
                for tt in range(TQ // P):
                    o_ps = psO.tile([P, C], F32, tag="o", name="o_ps")
                    tsl = slice(tt * P, (tt + 1) * P)
                    for k in range(KC):
                        for n in range(C // 512):
                            nsl = slice(n * 512, (n + 1) * 512)
                            nc.tensor.matmul(
                                o_ps[:, nsl], y_sb[k][:, tsl],
                                wc_sb[k][:, nsl],
                                start=(k == 0), stop=(k == KC - 1))
                    o_sb = pO.tile([P, C], F32, tag="o_sb", name="o_sb")
                    nc.vector.tensor_tensor(o_sb[:], o_ps[:], bb_sb[:], ALU.add)
                    nc.sync.dma_start(out[tsl, :], o_sb[:])

    nc.compile()
    _dedup_ldweights(nc)
    return nc


def _prep_maps(x, wq_d, bq_d, w_qk, b_qk, wkv_d, bkv_d, wv_u, bv_u, w_proj, b_proj):
    f = np.float32
    bf = ml_dtypes.bfloat16
    x = np.asarray(x, f)
    wq_d, bq_d = np.asarray(wq_d, f), np.asarray(bq_d, f)
    w_qk, b_qk = np.asarray(w_qk, f), np.asarray(b_qk, f)
    wkv_d, bkv_d = np.asarray(wkv_d, f), np.asarray(bkv_d, f)
    wv_u, bv_u = np.asarray(wv_u, f), np.asarray(bv_u, f)
    w_proj, b_proj = np.asarray(w_proj, f), np.asarray(b_proj, f)

    wc = np.concatenate(
        [w_proj[:, h * HD:(h + 1) * HD] @ wv_u[h * HD:(h + 1) * HD, :]
         for h in range(H)], axis=1)                    # [C, H*KV]
    bb = b_proj + w_proj @ bv_u                          # [C]
    bqk_f = w_qk @ bq_d + b_qk                           # [H*KV]

    shared = {
        "wq_dT": np.ascontiguousarray(wq_d.T).astype(bf),
        "w_qkT": np.ascontiguousarray(w_qk.T).astype(bf),
        "wkv_dT": np.ascontiguousarray(wkv_d.T).astype(bf),
        "wcT": np.ascontiguousarray(wc.T).astype(bf),
        "bqk_p": np.ascontiguousarray(bqk_f.reshape(MH, P).T),
        "bkv2": np.concatenate([bkv_d, bkv_d]).reshape(P, 1).astype(f),
        "bb_bc": np.broadcast_to(bb, (P, C)).copy(),
        "ones16_d": np.ones((P, 1), np.float16),
        "onesw_d": np.ones((P, 64), np.float16),
        "ident_d": np.eye(64, dtype=np.float16),
    }
    in_maps = []
    for c in range(8):
        b, half = divmod(c, 2)
        xTb = np.ascontiguousarray(x[b].T)               # [C, T]
        if half:
            xTb = np.ascontiguousarray(
                np.concatenate([xTb[:, TQ:], xTb[:, :TQ]], axis=1))
        m = dict(shared)
        m["xT"] = xTb.astype(bf)
        in_maps.append(m)
    return in_maps


def kernel(**inputs):
    if "nc" not in _CACHE:
        _CACHE["nc"] = build_nc()
    nc = _CACHE["nc"]
    in_maps = _prep_maps(**inputs)
    res = run_bass_kernel_spmd(nc, in_maps, core_ids=list(range(8)))
    _CACHE["last_result"] = res
    y = np.empty((B, T, C), dtype=np.float32)
    for c in range(8):
        b, half = divmod(c, 2)
        y[b, half * TQ:(half + 1) * TQ, :] = res.results[c]["out"]
    return y
